# revision 31
# baseline (speedup 1.0000x reference)
"""Multi-head attention (16 heads, RoPE, causal) for Trainium2, 8 NeuronCores.

Sharding: data-parallel over batch (2) x tensor-parallel over head groups (4),
one (batch, head-group-of-4) pair per core. Each core computes its 4 heads'
attention feature-major and a partial output projection outT [1024, 2048] in
fp16; the host sums the 4 partials per batch and transposes back.

v2 highlights over the bf16 baseline:
  - Q/K projections in fp8e4 DoubleRow perf mode (2 contraction planes per
    matmul, 0.5 cycles/col): host packs x and Wq/Wk into [128, 2, N] paired
    layouts with absmax-derived scales.
  - Q^T/K^T stored fp8 in an even/odd paired layout ([128 partitions =
    4 heads x 32 pairs] x [2 planes: even|odd] x 2048), so the S^T matmul is
    also fp8 DoubleRow with the pair dim as the second contraction plane.
  - RoPE runs on DVE directly from the projection PSUM; the fp8/bf16
    quantization scales and 1/sqrt(d_model) are folded into per-path
    cos/sin constant tables, and 1/(sq*sk) into the exp() activation scale.
  - Scalar (ACT) engine runs exp() only -- it is the critical resource
    (~80us of 1 elem/cycle/partition work).
  - GpSimd handles SBUF-only work: causal tri-mask multiplies, softmax
    denominator partition-broadcasts, V ones-column memsets.
  - Inputs stream over three DMA queues (sync: x fp8, act: weights/consts,
    gpsimd: x bf16); output is fp16 (half the store traffic).
  - A short warm-up matmul block keeps the PE busy during the DMA ramp so
    the pstate governor is part-way up before real work lands.
"""

import sys

sys.path.insert(0, "/opt/trn_rl_repo")
sys.path.insert(0, "/root/.axon_site")

import numpy as np

B, L, D = 2, 2048, 1024
H = 16                  # total heads
HD = 64                 # head dim
HPC = 4                 # heads per core
NCORES = 8
LC = L // 512           # 512-wide l chunks
KC = D // 128           # 128-deep contraction chunks (bf16 V path)
KC2 = D // 256          # 256-deep DoubleRow contraction chunks
LT = L // 128           # 128-row l tiles
NWARM = 20              # PE warm-up matmuls

_cache = {}


def _build_nc(causal: bool):
    import contextlib

    import concourse.bass as bass
    import concourse.tile as tile
    from concourse import bacc, mybir

    F32 = mybir.dt.float32
    BF16 = mybir.dt.bfloat16
    FP8 = mybir.dt.float8e4
    F16 = mybir.dt.float16
    EXP = mybir.ActivationFunctionType.Exp
    COPY = mybir.ActivationFunctionType.Copy
    DR = mybir.MatmulPerfMode.DoubleRow

    nc = bacc.Bacc("TRN2", target_bir_lowering=False, debug=False, num_devices=NCORES)

    # [16*128, 1024] tiles (lc, kc2): row p = [plane0 512 | plane1 512]
    x8 = nc.dram_tensor("x8", [16 * 128, 1024], FP8, kind="ExternalInput")
    xT = nc.dram_tensor("xT", [D, L], BF16, kind="ExternalInput")
    # [8*128, 256] tiles (kc2, nt): [plane0 128 feats | plane1 128]
    # wq8/wk8: straight feature order; wq8s/wk8s: pair-swapped columns so
    # rope is rot = cosq*P + srotq*Ps (no on-chip partition swap needed)
    wq8 = nc.dram_tensor("wq8", [8 * 128, 256], FP8, kind="ExternalInput")
    wq8s = nc.dram_tensor("wq8s", [8 * 128, 256], FP8, kind="ExternalInput")
    wk8 = nc.dram_tensor("wk8", [8 * 128, 256], FP8, kind="ExternalInput")
    wk8s = nc.dram_tensor("wk8s", [8 * 128, 256], FP8, kind="ExternalInput")
    wv = nc.dram_tensor("wv", [D, 256], BF16, kind="ExternalInput")
    wo = nc.dram_tensor("wo", [256, D], BF16, kind="ExternalInput")
    cosq = nc.dram_tensor("cosq", [128, L], BF16, kind="ExternalInput")
    sinq = nc.dram_tensor("sinq", [128, L], BF16, kind="ExternalInput")
    cosk = nc.dram_tensor("cosk", [128, L], BF16, kind="ExternalInput")
    sink = nc.dram_tensor("sink", [128, L], BF16, kind="ExternalInput")
    mk4 = nc.dram_tensor("mk4", [128, 128], BF16, kind="ExternalInput")
    outT = nc.dram_tensor("outT", [D, L], F16, kind="ExternalOutput")

    with tile.TileContext(nc) as tc, \
         nc.allow_low_precision(reason="fp8/bf16 matmul pipeline by design"), \
         contextlib.ExitStack() as ctx:
        p_w8 = ctx.enter_context(tc.tile_pool(name="p_w8", bufs=32))
        p_wv = ctx.enter_context(tc.tile_pool(name="p_wv", bufs=8))
        p_wo = ctx.enter_context(tc.tile_pool(name="p_wo", bufs=2))
        p_const = ctx.enter_context(tc.tile_pool(name="p_const", bufs=6))
        p_x8 = ctx.enter_context(tc.tile_pool(name="p_x8", bufs=16))
        p_xt = ctx.enter_context(tc.tile_pool(name="p_xt", bufs=8))
        p_qk = ctx.enter_context(tc.tile_pool(name="p_qk", bufs=4))
        p_yt = ctx.enter_context(tc.tile_pool(name="p_yt", bufs=2))
        p_v = ctx.enter_context(tc.tile_pool(name="p_v", bufs=16))
        p_pt = ctx.enter_context(tc.tile_pool(name="p_pt", bufs=13))
        p_tmp = ctx.enter_context(tc.tile_pool(name="p_tmp", bufs=6))
        p_zs = ctx.enter_context(tc.tile_pool(name="p_zs", bufs=4))
        p_zb = ctx.enter_context(tc.tile_pool(name="p_zb", bufs=3))
        p_oc = ctx.enter_context(tc.tile_pool(name="p_oc", bufs=6))
        p_wu = ctx.enter_context(tc.tile_pool(name="p_wu", bufs=1))
        pp = ctx.enter_context(tc.tile_pool(name="pp", bufs=2, space="PSUM"))
        pst = ctx.enter_context(tc.tile_pool(name="pst", bufs=2, space="PSUM"))
        pso = ctx.enter_context(tc.tile_pool(name="pso", bufs=2, space="PSUM"))

        # ---- warm-up: keep PE busy while DMAs land (pstate ramp).
        # Quarter-density (Pout=32) so the power-credit pool isn't drained
        # before the real work starts.
        wu = p_wu.tile([128, 512], BF16, tag="wu")
        nc.vector.memset(wu[:, :], 0.125)
        wu_ps = pp.tile([128, 512], F32, tag="pp")
        for _ in range(NWARM):
            nc.tensor.matmul(wu_ps[0:32, :], wu[:, 0:32], wu[:, :],
                             start=True, stop=True)

        # ---- input DMAs over two queues (ACT issues none: it is the
        # critical exp engine). Q-path inputs on sync, K-path on gpsimd so
        # the first S matmul's dependencies land in parallel. -------------
        x8_sb = {}

        def load_x8(lc, eng):
            for kc2 in range(KC2):
                t = p_x8.tile([128, 1024], FP8, tag="x8", name=f"x8_{lc}_{kc2}")
                r0 = (lc * KC2 + kc2) * 128
                eng.dma_start(out=t, in_=x8.ap()[r0:r0 + 128, :])
                x8_sb[(lc, kc2)] = t

        wq8_sb, wq8s_sb, wk8_sb, wk8s_sb = {}, {}, {}, {}

        def load_w8(srct, dst, eng):
            for kc2 in range(KC2):
                for nt in range(2):
                    t = p_w8.tile([128, 256], FP8, tag="w8")
                    r0 = (kc2 * 2 + nt) * 128
                    eng.dma_start(out=t, in_=srct.ap()[r0:r0 + 128, :])
                    dst[(kc2, nt)] = t

        rope_c = {}

        def load_const(nm, src, eng):
            t = p_const.tile([128, L], BF16, tag="const")
            eng.dma_start(out=t, in_=src.ap())
            rope_c[nm] = t

        xt_sb = [p_xt.tile([128, L], BF16, tag="xt", name=f"xt{kc}")
                 for kc in range(KC)]

        def load_xt(kc, eng):
            eng.dma_start(out=xt_sb[kc],
                          in_=xT.ap()[kc * 128:(kc + 1) * 128, :])

        # sync queue (Q path first, then V-path x)
        load_x8(0, nc.sync)
        load_w8(wq8, wq8_sb, nc.sync)
        load_w8(wq8s, wq8s_sb, nc.sync)
        load_const("cosq", cosq, nc.sync)
        load_const("sinq", sinq, nc.sync)
        load_x8(1, nc.sync)
        for kc in range(4):
            load_xt(kc, nc.sync)
        load_x8(2, nc.sync)
        load_x8(3, nc.sync)
        wv_sb = []
        for kc in range(KC):
            t = p_wv.tile([128, 256], BF16, tag="wv")
            nc.sync.dma_start(out=t, in_=wv.ap()[kc * 128:(kc + 1) * 128, :])
            wv_sb.append(t)
        # gpsimd queue (K path first, then remaining V-path x)
        mk_t = p_const.tile([128, 128], BF16, tag="tri")
        nc.gpsimd.dma_start(out=mk_t, in_=mk4.ap())
        load_w8(wk8, wk8_sb, nc.gpsimd)
        load_w8(wk8s, wk8s_sb, nc.gpsimd)
        load_const("cosk", cosk, nc.gpsimd)
        load_const("sink", sink, nc.gpsimd)
        for kc in range(4, KC):
            load_xt(kc, nc.gpsimd)
        wo_sb = []
        for kc2 in range(2):
            t = p_wo.tile([128, D], BF16, tag="wo")
            nc.gpsimd.dma_start(out=t, in_=wo.ap()[kc2 * 128:(kc2 + 1) * 128, :])
            wo_sb.append(t)

        # persistent activation tiles: bf16 Q^T/K^T, 2 heads per nt tile,
        # rows h*64+u with u<32 = even rotary dims, u>=32 = odd dims
        qt_sb = [p_qk.tile([128, L], BF16, tag="qt", name=f"qt{i}")
                 for i in range(2)]
        kt_sb = [p_qk.tile([128, L], BF16, tag="kt", name=f"kt{i}")
                 for i in range(2)]
        yt_sb = [p_yt.tile([128, L], BF16, tag="yt", name=f"yt{i}")
                 for i in range(2)]
        v_sb = [p_v.tile([128, HPC, 65], BF16, tag="vaug", name=f"vaug{i}")
                for i in range(LT)]
        for lt in range(LT):
            nc.gpsimd.memset(v_sb[lt][:, :, 64:65], 1.0)

        # ---- QK projection (fp8 DoubleRow x2) + RoPE -------------------
        # Two parallel projections per tile: P = x@W (straight features)
        # and Ps = x@Ws (pair-swapped features), then
        # rope(q) = cos*P + srot*Ps with per-row-signed srot.
        def proj_chunk(w_sb, ws_sb, trg, lc, cosn, sinn):
            csl = slice(lc * 512, (lc + 1) * 512)
            cos_t, sin_t = rope_c[cosn], rope_c[sinn]
            for nt in range(2):
                ps1 = pp.tile([128, 512], F32, tag="pp")
                ps2 = pp.tile([128, 512], F32, tag="pp")
                for dst, wmap in ((ps1, w_sb), (ps2, ws_sb)):
                    for kc2 in range(KC2):
                        nc.tensor.matmul(
                            dst[:, :],
                            wmap[(kc2, nt)][:, :].rearrange(
                                "p (two m) -> p two m", two=2),
                            x8_sb[(lc, kc2)][:, :].rearrange(
                                "p (two n) -> p two n", two=2),
                            start=(kc2 == 0), stop=(kc2 == KC2 - 1),
                            perf_mode=DR)
                m1 = p_tmp.tile([128, 512], BF16, tag="tmp")
                nc.vector.tensor_mul(m1[:, :], ps1[:, :], cos_t[:, csl])
                m2 = p_tmp.tile([128, 512], BF16, tag="tmp")
                nc.vector.tensor_mul(m2[:, :], ps2[:, :], sin_t[:, csl])
                nc.vector.tensor_add(trg[nt][:, csl], m1[:, :], m2[:, :])

        def proj_qk(lc):
            proj_chunk(wq8_sb, wq8s_sb, qt_sb, lc, "cosq", "sinq")
            proj_chunk(wk8_sb, wk8s_sb, kt_sb, lc, "cosk", "sink")

        # ---- V tile (bf16) ---------------------------------------------
        def v_tile(lt):
            ps = pp.tile([128, 256], F32, tag="pp")
            for kc in range(KC):
                nc.tensor.matmul(
                    ps[:, :], xt_sb[kc][:, lt * 128:(lt + 1) * 128],
                    wv_sb[kc][:, :], start=(kc == 0), stop=(kc == KC - 1))
            nc.vector.tensor_copy(
                v_sb[lt][:, :, 0:64],
                ps[:, :].rearrange("p (h v) -> p h v", h=HPC))

        # ---- attention -------------------------------------------------
        exp_scale = float(_EXP_SCALE[0])
        pending_yt = []   # deferred normalize muls (DVE must not stall on
                          # the gpsimd broadcast latency)

        def flush_yt():
            while pending_yt:
                oaug, zb, nt, r0, csl = pending_yt.pop(0)
                nc.vector.tensor_mul(yt_sb[nt][r0:r0 + 64, csl],
                                     oaug[0:64, :], zb[:, :])

        def normalize(oaug, nt, r0, csl):
            zs = p_zs.tile([1, 512], F32, tag="zs")
            nc.vector.tensor_copy(zs[0:1, :], oaug[64:65, :])
            zrow = p_zs.tile([1, 512], F32, tag="zrow")
            nc.vector.reciprocal_approx_fast(zrow[0:1, :], zs[0:1, :])
            zb = p_zb.tile([64, 512], F32, tag="zb")
            nc.gpsimd.partition_broadcast(zb[:, :], zrow[0:1, :])
            flush_yt()
            pending_yt.append((oaug, zb, nt, r0, csl))

        def trim(c, j):
            k = j - 4 * c
            return 128 * k if (causal and k >= 0) else 0

        def att_se(c, h):
            """S matmuls + exp for all pairs of (c, h); returns pt list."""
            nt, r0 = h // 2, (h % 2) * 64
            jmax = 4 * c + 3 if causal else LT - 1
            pts = []
            for jp in range((jmax + 1) // 2):
                st = pst.tile([128, 1024], F32, tag="st")
                for s in range(2):
                    j = 2 * jp + s
                    t = trim(c, j)
                    nc.tensor.matmul(
                        st[:, s * 512 + t:(s + 1) * 512],
                        kt_sb[nt][r0:r0 + 64, j * 128:(j + 1) * 128],
                        qt_sb[nt][r0:r0 + 64, c * 512 + t:(c + 1) * 512],
                        start=True, stop=True)
                pt = p_pt.tile([128, 1024], BF16, tag="pt")
                t0 = trim(c, 2 * jp)
                nc.scalar.activation(pt[:, t0:], st[:, t0:], EXP,
                                     scale=exp_scale)
                if causal:
                    for s in range(2):
                        k = 2 * jp + s - 4 * c
                        if k >= 0:
                            sl = slice(s * 512 + 128 * k,
                                       s * 512 + 128 * (k + 1))
                            nc.vector.tensor_mul(pt[:, sl], pt[:, sl],
                                                 mk_t[:, :])
                pts.append((jp, pt))
            return pts

        def att_o(c, h, pts):
            """O accumulation + raw evac + recip/broadcast; defers yt mul."""
            nt, r0 = h // 2, (h % 2) * 64
            csl = slice(c * 512, (c + 1) * 512)
            jmax = 4 * c + 3 if causal else LT - 1
            oaug = pso.tile([65, 512], F32, tag="oaug")
            for jp, pt in pts:
                for s in range(2):
                    j = 2 * jp + s
                    t = trim(c, j)
                    nc.tensor.matmul(
                        oaug[:, t:512], v_sb[j][:, h, :],
                        pt[:, s * 512 + t:(s + 1) * 512],
                        start=(j == 0), stop=(j == jmax))
            normalize(oaug, nt, r0, csl)

        def att_full(c, h, lag=2):
            nt, r0 = h // 2, (h % 2) * 64
            csl = slice(c * 512, (c + 1) * 512)
            jmax = 4 * c + 3 if causal else LT - 1
            oaug = pso.tile([65, 512], F32, tag="oaug")

            def emit_o(jp, pt):
                for s in range(2):
                    j = 2 * jp + s
                    t = trim(c, j)
                    nc.tensor.matmul(
                        oaug[:, t:512], v_sb[j][:, h, :],
                        pt[:, s * 512 + t:(s + 1) * 512],
                        start=(j == 0), stop=(j == jmax))

            lagq = []
            for jp in range((jmax + 1) // 2):
                st = pst.tile([128, 1024], F32, tag="st")
                for s in range(2):
                    j = 2 * jp + s
                    t = trim(c, j)
                    nc.tensor.matmul(
                        st[:, s * 512 + t:(s + 1) * 512],
                        kt_sb[nt][r0:r0 + 64, j * 128:(j + 1) * 128],
                        qt_sb[nt][r0:r0 + 64, c * 512 + t:(c + 1) * 512],
                        start=True, stop=True)
                pt = p_pt.tile([128, 1024], BF16, tag="pt")
                t0 = trim(c, 2 * jp)
                nc.scalar.activation(pt[:, t0:], st[:, t0:], EXP,
                                     scale=exp_scale)
                if causal:
                    for s in range(2):
                        k = 2 * jp + s - 4 * c
                        if k >= 0:
                            sl = slice(s * 512 + 128 * k,
                                       s * 512 + 128 * (k + 1))
                            nc.vector.tensor_mul(pt[:, sl], pt[:, sl],
                                                 mk_t[:, :])
                lagq.append((jp, pt))
                if len(lagq) > lag:
                    emit_o(*lagq.pop(0))
            for args in lagq:
                emit_o(*args)
            normalize(oaug, nt, r0, csl)

        # ---- output projection pieces -----------------------------------
        def wo_piece(c, ots, tail=False):
            flush_yt()
            for ot in ots:
                ps = pp.tile([128, 512], F32, tag="pp")
                for kc2 in range(2):
                    nc.tensor.matmul(
                        ps[:, :], wo_sb[kc2][:, ot * 128:(ot + 1) * 128],
                        yt_sb[kc2][:, c * 512:(c + 1) * 512],
                        start=(kc2 == 0), stop=(kc2 == 1))
                oc = p_oc.tile([128, 512], F16, tag="oc")
                if tail:
                    nc.scalar.activation(oc[:, :], ps[:, :], COPY)
                else:
                    nc.vector.tensor_copy(oc[:, :], ps[:, :])
                nc.sync.dma_start(
                    out=outT.ap()[ot * 128:(ot + 1) * 128,
                                  c * 512:(c + 1) * 512],
                    in_=oc[:, :])

        # ---- emission schedule ------------------------------------------
        # exp stream starts as early as possible; V-dependent O work is
        # emitted after each chunk's S/exp block so late xt DMAs cannot
        # stall the in-order tensor queue ahead of the exps.
        proj_qk(0)
        # att(0): S/exp for all 4 heads first (8 pairs buffered in p_pt)
        pts0 = [att_se(0, h) for h in range(2)]
        proj_chunk(wq8_sb, wq8s_sb, qt_sb, 1, "cosq", "sinq")
        pts0 += [att_se(0, h) for h in range(2, 4)]
        proj_chunk(wk8_sb, wk8s_sb, kt_sb, 1, "cosk", "sink")
        for lt in range(4):
            v_tile(lt)
        for h in range(HPC):
            att_o(0, h, pts0[h])
        # att(1)
        pts10 = att_se(1, 0)
        for lt in range(4, 8):
            v_tile(lt)
        att_o(1, 0, pts10)
        proj_chunk(wq8_sb, wq8s_sb, qt_sb, 2, "cosq", "sinq")
        att_full(1, 1)
        proj_chunk(wk8_sb, wk8s_sb, kt_sb, 2, "cosk", "sink")
        att_full(1, 2)
        att_full(1, 3)
        # att(2)
        pts20 = att_se(2, 0)
        for lt in range(8, 12):
            v_tile(lt)
        att_o(2, 0, pts20)
        wo_piece(0, range(0, 4))
        att_full(2, 1)
        proj_chunk(wq8_sb, wq8s_sb, qt_sb, 3, "cosq", "sinq")
        att_full(2, 2)
        proj_chunk(wk8_sb, wk8s_sb, kt_sb, 3, "cosk", "sink")
        att_full(2, 3)
        wo_piece(0, range(4, 8))
        # att(3)
        pts30 = att_se(3, 0)
        for lt in range(12, 16):
            v_tile(lt)
        att_o(3, 0, pts30)
        wo_piece(1, range(0, 4))
        att_full(3, 1)
        wo_piece(1, range(4, 8))
        att_full(3, 2)
        wo_piece(2, range(0, 8))
        att_full(3, 3)
        wo_piece(3, range(8), tail=True)

    nc.compile()
    return nc


_EXP_SCALE = [1.0]


def _get_nc(causal: bool, exp_scale: float):
    key = ("causal" if causal else "dense", round(float(exp_scale), 12))
    if key not in _cache:
        _EXP_SCALE[0] = float(exp_scale)
        _cache[key] = _build_nc(causal)
    return _cache[key]


def _rope_np(x):
    d, s = x.shape[-1], x.shape[-2]
    ts = np.arange(0, d, 2, dtype=np.float32)
    inv = 10000.0 ** (-ts / d)
    grid = np.arange(s, dtype=np.float32)[:, None] * inv[None, :]
    sin = np.repeat(np.sin(grid), 2, axis=-1)
    cos = np.repeat(np.cos(grid), 2, axis=-1)
    x1, x2 = x[..., ::2], x[..., 1::2]
    xs = np.stack([-x2, x1], axis=-1).reshape(x.shape)
    return x * cos + xs * sin


def _reference_np(x, mask, Wq, Wk, Wv, Wo):
    b, l, d = x.shape
    h, k_sz = H, D // H
    split = lambda t: t.reshape(b, l, h, k_sz).transpose(0, 2, 1, 3)
    q = split((x @ Wq) / np.sqrt(np.float32(d)))
    q = _rope_np(q)
    k = _rope_np(split(x @ Wk))
    v = split(x @ Wv)
    logits = np.einsum("bhik,bhjk->bhij", q, k) + mask
    m = logits.max(axis=-1, keepdims=True)
    p = np.exp(logits - m)
    a = p / p.sum(axis=-1, keepdims=True)
    y = np.einsum("bhij,bhjv->bhiv", a, v)
    y = y.transpose(0, 2, 1, 3).reshape(b, l, d)
    return (y @ Wo).astype(np.float32)


def _spectral_norm(w, iters=12):
    rng = np.random.default_rng(0)
    v = rng.standard_normal(w.shape[1]).astype(np.float32)
    for _ in range(iters):
        u = w @ v
        u /= (np.linalg.norm(u) + 1e-30)
        v = w.T @ u
        nv = np.linalg.norm(v)
        v /= (nv + 1e-30)
    return float(nv)


def _host_consts():
    inv = 10000.0 ** (-np.arange(0, HD, 2, dtype=np.float32) / HD)
    grid = np.arange(L, dtype=np.float32)[None, :] * inv[:, None]   # [32, L]
    cos32 = np.cos(grid).astype(np.float32)
    sin32 = np.sin(grid).astype(np.float32)
    cos128 = np.ascontiguousarray(np.tile(cos32, (4, 1)))
    # srot rows u: u%64<32 (even-dim rows) get -sin, u%64>=32 get +sin
    srot128 = np.ascontiguousarray(
        np.tile(np.concatenate([-sin32, sin32], axis=0), (2, 1)))
    tri = (np.arange(128)[None, :] >= np.arange(128)[:, None]).astype(np.float32)
    return cos128, srot128, np.ascontiguousarray(tri)


def _pack_dr_w(Wcols, scale, e4):
    """Wcols [1024, 256] (E feats 0:128, O feats 128:256) ->
    [8*128, 256] fp8 DoubleRow tiles (kc2, nt)."""
    out = np.empty((8 * 128, 256), np.float32)
    for kc2 in range(KC2):
        for nt in range(2):
            r0 = (kc2 * 2 + nt) * 128
            blk = Wcols[kc2 * 256:(kc2 + 1) * 256, nt * 128:(nt + 1) * 128]
            out[r0:r0 + 128, 0:128] = blk[0:128]
            out[r0:r0 + 128, 128:256] = blk[128:256]
    return np.clip(out * scale, -240, 240).astype(e4)


def _make_in_maps(x, Wq, Wk, Wv, Wo):
    import ml_dtypes
    bf16 = ml_dtypes.bfloat16
    e4 = ml_dtypes.float8_e4m3

    cos128, srot128, tri = _host_consts()

    sx = 240.0 / max(float(np.abs(x).max()), 1e-30)
    swq = 240.0 / max(float(np.abs(Wq).max()), 1e-30)
    swk = 240.0 / max(float(np.abs(Wk).max()), 1e-30)
    # fold the fp8 scales (and q's 1/sqrt(d_model)) into the rope tables so
    # qt/kt come out in true units and the exp scale is exactly 1
    fq = 1.0 / (sx * swq * float(np.sqrt(np.float32(D))))
    fk = 1.0 / (sx * swk)
    exp_scale = 1.0

    cosq = np.ascontiguousarray((cos128 * fq).astype(bf16))
    sinq = np.ascontiguousarray((srot128 * fq).astype(bf16))
    cosk = np.ascontiguousarray((cos128 * fk).astype(bf16))
    sink = np.ascontiguousarray((srot128 * fk).astype(bf16))
    mk4 = tri.astype(bf16)

    in_maps = []
    for core in range(NCORES):
        bi, g = core // 4, core % 4
        xTb = x[bi].T  # [1024, 2048] f32
        # fp8 DR x tiles: (lc, kc2) [128, 1024]
        x8 = np.empty((16 * 128, 1024), np.float32)
        for lc in range(LC):
            for kc2 in range(KC2):
                r0 = (lc * KC2 + kc2) * 128
                lsl = slice(lc * 512, (lc + 1) * 512)
                x8[r0:r0 + 128, 0:512] = xTb[kc2 * 256:kc2 * 256 + 128, lsl]
                x8[r0:r0 + 128, 512:1024] = xTb[kc2 * 256 + 128:
                                                kc2 * 256 + 256, lsl]
        x8 = np.clip(x8 * sx, -240, 240).astype(e4)

        # feature column orders: straight = per head [even dims | odd dims],
        # swapped = per head [odd dims | even dims] (rope pair partners)
        cols, cols_s = [], []
        for hh in range(HPC):
            base = (g * HPC + hh) * 64
            ev = list(range(base, base + 64, 2))
            od = list(range(base + 1, base + 64, 2))
            cols.extend(ev + od)
            cols_s.extend(od + ev)
        in_maps.append({
            "x8": x8,
            "xT": np.ascontiguousarray(xTb.astype(bf16)),
            "wq8": _pack_dr_w(Wq[:, cols], swq, e4),
            "wq8s": _pack_dr_w(Wq[:, cols_s], swq, e4),
            "wk8": _pack_dr_w(Wk[:, cols], swk, e4),
            "wk8s": _pack_dr_w(Wk[:, cols_s], swk, e4),
            "wv": np.ascontiguousarray(
                Wv[:, g * 256:(g + 1) * 256].astype(bf16)),
            "wo": np.ascontiguousarray(
                Wo[g * 256:(g + 1) * 256, :].astype(bf16)),
            "cosq": cosq, "sinq": sinq, "cosk": cosk, "sink": sink,
            "mk4": mk4,
        })
    return in_maps, exp_scale


def kernel(x, mask, Wq, Wk, Wv, Wo):
    from concourse.bass_utils import run_bass_kernel_spmd

    x = np.asarray(x, dtype=np.float32)
    mask = np.asarray(mask, dtype=np.float32)
    Wq = np.asarray(Wq, dtype=np.float32)
    Wk = np.asarray(Wk, dtype=np.float32)
    Wv = np.asarray(Wv, dtype=np.float32)
    Wo = np.asarray(Wo, dtype=np.float32)

    m = mask.reshape(L, L)
    tril = np.tril(np.ones((L, L), dtype=bool))
    visible = m > -1e6
    if np.array_equal(visible, tril) and not m[tril].any():
        causal = True
    elif not m.any():
        causal = False
    else:
        return _reference_np(x, mask, Wq, Wk, Wv, Wo)

    # overflow guard for the no-max-subtraction softmax
    xr = float(np.sqrt((x * x).sum(axis=2).max()))
    bound = (xr * _spectral_norm(Wq) / np.sqrt(D)) * (xr * _spectral_norm(Wk))
    if bound > 60.0:
        return _reference_np(x, mask, Wq, Wk, Wv, Wo)

    in_maps, exp_scale = _make_in_maps(x, Wq, Wk, Wv, Wo)
    nc = _get_nc(causal, exp_scale)
    res = run_bass_kernel_spmd(nc, in_maps, core_ids=list(range(NCORES)))

    out = np.empty((B, L, D), dtype=np.float32)
    for bi in range(B):
        acc = res.results[bi * 4]["outT"].astype(np.float32)
        for g in range(1, 4):
            acc += res.results[bi * 4 + g]["outT"].astype(np.float32)
        out[bi] = acc.T
    return out


# revision 38
# speedup vs baseline: 1.0701x; 1.0701x over previous
"""Multi-head attention (16 heads, RoPE, causal) for Trainium2, 8 NeuronCores.

Sharding: data-parallel over batch (2) x tensor-parallel over head groups (4),
one (batch, head-group-of-4) pair per core. Each core computes its 4 heads'
attention feature-major and a partial output projection outT [1024, 2048] in
fp16; the host sums the 4 partials per batch and transposes back.

v2 highlights over the bf16 baseline:
  - Q/K projections in fp8e4 DoubleRow perf mode (2 contraction planes per
    matmul, 0.5 cycles/col): host packs x and Wq/Wk into [128, 2, N] paired
    layouts with absmax-derived scales.
  - Q^T/K^T stored fp8 in an even/odd paired layout ([128 partitions =
    4 heads x 32 pairs] x [2 planes: even|odd] x 2048), so the S^T matmul is
    also fp8 DoubleRow with the pair dim as the second contraction plane.
  - RoPE runs on DVE directly from the projection PSUM; the fp8/bf16
    quantization scales and 1/sqrt(d_model) are folded into per-path
    cos/sin constant tables, and 1/(sq*sk) into the exp() activation scale.
  - Scalar (ACT) engine runs exp() only -- it is the critical resource
    (~80us of 1 elem/cycle/partition work).
  - GpSimd handles SBUF-only work: causal tri-mask multiplies, softmax
    denominator partition-broadcasts, V ones-column memsets.
  - Inputs stream over three DMA queues (sync: x fp8, act: weights/consts,
    gpsimd: x bf16); output is fp16 (half the store traffic).
  - A short warm-up matmul block keeps the PE busy during the DMA ramp so
    the pstate governor is part-way up before real work lands.
"""

import sys

sys.path.insert(0, "/opt/trn_rl_repo")
sys.path.insert(0, "/root/.axon_site")

import numpy as np

B, L, D = 2, 2048, 1024
H = 16                  # total heads
HD = 64                 # head dim
HPC = 4                 # heads per core
NCORES = 8
LC = L // 512           # 512-wide l chunks
KC = D // 128           # 128-deep contraction chunks (bf16 V path)
KC2 = D // 256          # 256-deep DoubleRow contraction chunks
LT = L // 128           # 128-row l tiles
NWARM = 20              # PE warm-up matmuls

_cache = {}


def _build_nc(causal: bool):
    import contextlib

    import concourse.bass as bass
    import concourse.tile as tile
    from concourse import bacc, mybir

    F32 = mybir.dt.float32
    BF16 = mybir.dt.bfloat16
    FP8 = mybir.dt.float8e4
    F16 = mybir.dt.float16
    EXP = mybir.ActivationFunctionType.Exp
    COPY = mybir.ActivationFunctionType.Copy
    DR = mybir.MatmulPerfMode.DoubleRow

    nc = bacc.Bacc("TRN2", target_bir_lowering=False, debug=False, num_devices=NCORES)

    # x fp8 packed per lc chunk: [128, kc2*1024 + plane*512 + n] (4KB rows)
    x8 = nc.dram_tensor("x8", [4 * 128, 4096], FP8, kind="ExternalInput")
    xT = nc.dram_tensor("xT", [D, L], BF16, kind="ExternalInput")
    # all Q (or K) DoubleRow weights in one 4KB-row tensor:
    # col block (variant*8 + kc2*2 + nt)*256 + plane*128 + m
    # variant 0 = straight feature order, 1 = pair-swapped columns so
    # rope is rot = cosq*P + srotq*Ps (no on-chip partition swap needed)
    wq8 = nc.dram_tensor("wq8", [128, 4096], FP8, kind="ExternalInput")
    wk8 = nc.dram_tensor("wk8", [128, 4096], FP8, kind="ExternalInput")
    # wv packed: col block kc*256 + v  (4KB rows bf16)
    wv = nc.dram_tensor("wv", [128, 8 * 256], BF16, kind="ExternalInput")
    wo = nc.dram_tensor("wo", [256, D], BF16, kind="ExternalInput")
    cosq = nc.dram_tensor("cosq", [128, L], BF16, kind="ExternalInput")
    sinq = nc.dram_tensor("sinq", [128, L], BF16, kind="ExternalInput")
    cosk = nc.dram_tensor("cosk", [128, L], BF16, kind="ExternalInput")
    sink = nc.dram_tensor("sink", [128, L], BF16, kind="ExternalInput")
    mk4 = nc.dram_tensor("mk4", [128, 128], BF16, kind="ExternalInput")
    outT = nc.dram_tensor("outT", [D, L], F16, kind="ExternalOutput")

    with tile.TileContext(nc) as tc, \
         nc.allow_low_precision(reason="fp8/bf16 matmul pipeline by design"), \
         contextlib.ExitStack() as ctx:
        p_w8 = ctx.enter_context(tc.tile_pool(name="p_w8", bufs=2))
        p_wv = ctx.enter_context(tc.tile_pool(name="p_wv", bufs=1))
        p_wo = ctx.enter_context(tc.tile_pool(name="p_wo", bufs=2))
        p_const = ctx.enter_context(tc.tile_pool(name="p_const", bufs=6))
        p_x8 = ctx.enter_context(tc.tile_pool(name="p_x8", bufs=4))
        p_xt = ctx.enter_context(tc.tile_pool(name="p_xt", bufs=8))
        p_qk = ctx.enter_context(tc.tile_pool(name="p_qk", bufs=4))
        p_yt = ctx.enter_context(tc.tile_pool(name="p_yt", bufs=2))
        p_v = ctx.enter_context(tc.tile_pool(name="p_v", bufs=16))
        p_pt = ctx.enter_context(tc.tile_pool(name="p_pt", bufs=13))
        p_tmp = ctx.enter_context(tc.tile_pool(name="p_tmp", bufs=6))
        p_zs = ctx.enter_context(tc.tile_pool(name="p_zs", bufs=4))
        p_zb = ctx.enter_context(tc.tile_pool(name="p_zb", bufs=3))
        p_oc = ctx.enter_context(tc.tile_pool(name="p_oc", bufs=6))
        p_wu = ctx.enter_context(tc.tile_pool(name="p_wu", bufs=1))
        pp = ctx.enter_context(tc.tile_pool(name="pp", bufs=2, space="PSUM"))
        pst = ctx.enter_context(tc.tile_pool(name="pst", bufs=2, space="PSUM"))
        pso = ctx.enter_context(tc.tile_pool(name="pso", bufs=2, space="PSUM"))

        # ---- warm-up: keep PE busy while DMAs land (pstate ramp).
        # Quarter-density (Pout=32) so the power-credit pool isn't drained
        # before the real work starts.
        wu = p_wu.tile([128, 512], BF16, tag="wu")
        nc.vector.memset(wu[:, :], 0.125)
        wu_ps = pp.tile([128, 512], F32, tag="pp")
        for _ in range(NWARM):
            nc.tensor.matmul(wu_ps[0:32, :], wu[:, 0:32], wu[:, :],
                             start=True, stop=True)

        # ---- input DMAs over two queues (ACT issues none: it is the
        # critical exp engine). Q-path inputs on sync, K-path on gpsimd so
        # the first S matmul's dependencies land in parallel. All inputs
        # are packed host-side into 4KB-row tensors for fat descriptors. --
        x8_sb = {}

        def load_x8(lc, eng):
            t = p_x8.tile([128, 4096], FP8, tag="x8", name=f"x8_{lc}")
            eng.dma_start(out=t, in_=x8.ap()[lc * 128:(lc + 1) * 128, :])
            x8_sb[lc] = t

        rope_c = {}

        def load_const(nm, src, eng):
            t = p_const.tile([128, L], BF16, tag="const")
            eng.dma_start(out=t, in_=src.ap())
            rope_c[nm] = t

        xt_sb = [p_xt.tile([128, L], BF16, tag="xt", name=f"xt{kc}")
                 for kc in range(KC)]

        def load_xt(kc, eng):
            eng.dma_start(out=xt_sb[kc],
                          in_=xT.ap()[kc * 128:(kc + 1) * 128, :])

        # sync queue (Q path first, then V-path x)
        load_x8(0, nc.sync)
        wq8_t = p_w8.tile([128, 4096], FP8, tag="w8")
        nc.sync.dma_start(out=wq8_t, in_=wq8.ap())
        load_const("cosq", cosq, nc.sync)
        load_const("sinq", sinq, nc.sync)
        load_x8(1, nc.sync)
        for kc in range(4):
            load_xt(kc, nc.sync)
        load_x8(2, nc.sync)
        load_x8(3, nc.sync)
        wv_t = p_wv.tile([128, 8 * 256], BF16, tag="wv")
        nc.sync.dma_start(out=wv_t, in_=wv.ap())
        # gpsimd queue (K path first, then remaining V-path x)
        mk_t = p_const.tile([128, 128], BF16, tag="tri")
        nc.gpsimd.dma_start(out=mk_t, in_=mk4.ap())
        wk8_t = p_w8.tile([128, 4096], FP8, tag="w8")
        nc.gpsimd.dma_start(out=wk8_t, in_=wk8.ap())
        load_const("cosk", cosk, nc.gpsimd)
        load_const("sink", sink, nc.gpsimd)
        for kc in range(4, KC):
            load_xt(kc, nc.gpsimd)
        wo_sb = []
        for kc2 in range(2):
            t = p_wo.tile([128, D], BF16, tag="wo")
            nc.gpsimd.dma_start(out=t, in_=wo.ap()[kc2 * 128:(kc2 + 1) * 128, :])
            wo_sb.append(t)

        # persistent activation tiles: bf16 Q^T/K^T, 2 heads per nt tile,
        # rows h*64+u with u<32 = even rotary dims, u>=32 = odd dims
        qt_sb = [p_qk.tile([128, L], BF16, tag="qt", name=f"qt{i}")
                 for i in range(2)]
        kt_sb = [p_qk.tile([128, L], BF16, tag="kt", name=f"kt{i}")
                 for i in range(2)]
        yt_sb = [p_yt.tile([128, L], BF16, tag="yt", name=f"yt{i}")
                 for i in range(2)]
        v_sb = [p_v.tile([128, HPC, 65], BF16, tag="vaug", name=f"vaug{i}")
                for i in range(LT)]
        for lt in range(LT):
            nc.gpsimd.memset(v_sb[lt][:, :, 64:65], 1.0)

        # ---- QK projection (fp8 DoubleRow x2) + RoPE -------------------
        # Two parallel projections per tile: P = x@W (straight features)
        # and Ps = x@Ws (pair-swapped features), then
        # rope(q) = cos*P + srot*Ps with per-row-signed srot.
        def proj_chunk(w_t, trg, lc, cosn, sinn):
            csl = slice(lc * 512, (lc + 1) * 512)
            cos_t, sin_t = rope_c[cosn], rope_c[sinn]
            for nt in range(2):
                ps1 = pp.tile([128, 512], F32, tag="pp")
                ps2 = pp.tile([128, 512], F32, tag="pp")
                for dst, var in ((ps1, 0), (ps2, 1)):
                    for kc2 in range(KC2):
                        woff = (var * 8 + kc2 * 2 + nt) * 256
                        nc.tensor.matmul(
                            dst[:, :],
                            w_t[:, woff:woff + 256].rearrange(
                                "p (two m) -> p two m", two=2),
                            x8_sb[lc][:, kc2 * 1024:(kc2 + 1) * 1024
                                      ].rearrange(
                                "p (two n) -> p two n", two=2),
                            start=(kc2 == 0), stop=(kc2 == KC2 - 1),
                            perf_mode=DR)
                m1 = p_tmp.tile([128, 512], BF16, tag="tmp")
                nc.vector.tensor_mul(m1[:, :], ps1[:, :], cos_t[:, csl])
                m2 = p_tmp.tile([128, 512], BF16, tag="tmp")
                nc.vector.tensor_mul(m2[:, :], ps2[:, :], sin_t[:, csl])
                nc.vector.tensor_add(trg[nt][:, csl], m1[:, :], m2[:, :])

        def proj_qk(lc):
            proj_chunk(wq8_t, qt_sb, lc, "cosq", "sinq")
            proj_chunk(wk8_t, kt_sb, lc, "cosk", "sink")

        # ---- V tile (bf16) ---------------------------------------------
        def v_tile(lt):
            ps = pp.tile([128, 256], F32, tag="pp")
            for kc in range(KC):
                nc.tensor.matmul(
                    ps[:, :], xt_sb[kc][:, lt * 128:(lt + 1) * 128],
                    wv_t[:, kc * 256:(kc + 1) * 256],
                    start=(kc == 0), stop=(kc == KC - 1))
            nc.vector.tensor_copy(
                v_sb[lt][:, :, 0:64],
                ps[:, :].rearrange("p (h v) -> p h v", h=HPC))

        # ---- attention -------------------------------------------------
        exp_scale = float(_EXP_SCALE[0])
        pending_yt = []   # deferred normalize muls (DVE must not stall on
                          # the gpsimd broadcast latency)

        def flush_yt():
            while pending_yt:
                oaug, zb, nt, r0, csl = pending_yt.pop(0)
                nc.vector.tensor_mul(yt_sb[nt][r0:r0 + 64, csl],
                                     oaug[0:64, :], zb[:, :])

        def normalize(oaug, nt, r0, csl):
            zs = p_zs.tile([1, 512], F32, tag="zs")
            nc.vector.tensor_copy(zs[0:1, :], oaug[64:65, :])
            zrow = p_zs.tile([1, 512], F32, tag="zrow")
            nc.vector.reciprocal_approx_fast(zrow[0:1, :], zs[0:1, :])
            zb = p_zb.tile([64, 512], F32, tag="zb")
            nc.gpsimd.partition_broadcast(zb[:, :], zrow[0:1, :])
            flush_yt()
            pending_yt.append((oaug, zb, nt, r0, csl))

        def trim(c, j):
            k = j - 4 * c
            return 128 * k if (causal and k >= 0) else 0

        def att_se(c, h):
            """S matmuls + exp for all pairs of (c, h); returns pt list."""
            nt, r0 = h // 2, (h % 2) * 64
            jmax = 4 * c + 3 if causal else LT - 1
            pts = []
            for jp in range((jmax + 1) // 2):
                st = pst.tile([128, 1024], F32, tag="st")
                for s in range(2):
                    j = 2 * jp + s
                    t = trim(c, j)
                    nc.tensor.matmul(
                        st[:, s * 512 + t:(s + 1) * 512],
                        kt_sb[nt][r0:r0 + 64, j * 128:(j + 1) * 128],
                        qt_sb[nt][r0:r0 + 64, c * 512 + t:(c + 1) * 512],
                        start=True, stop=True)
                pt = p_pt.tile([128, 1024], BF16, tag="pt")
                t0 = trim(c, 2 * jp)
                nc.scalar.activation(pt[:, t0:], st[:, t0:], EXP,
                                     scale=exp_scale)
                if causal:
                    for s in range(2):
                        k = 2 * jp + s - 4 * c
                        if k >= 0:
                            sl = slice(s * 512 + 128 * k,
                                       s * 512 + 128 * (k + 1))
                            nc.vector.tensor_mul(pt[:, sl], pt[:, sl],
                                                 mk_t[:, :])
                pts.append((jp, pt))
            return pts

        def att_o(c, h, pts):
            """O accumulation + raw evac + recip/broadcast; defers yt mul."""
            nt, r0 = h // 2, (h % 2) * 64
            csl = slice(c * 512, (c + 1) * 512)
            jmax = 4 * c + 3 if causal else LT - 1
            oaug = pso.tile([65, 512], F32, tag="oaug")
            for jp, pt in pts:
                for s in range(2):
                    j = 2 * jp + s
                    t = trim(c, j)
                    nc.tensor.matmul(
                        oaug[:, t:512], v_sb[j][:, h, :],
                        pt[:, s * 512 + t:(s + 1) * 512],
                        start=(j == 0), stop=(j == jmax))
            normalize(oaug, nt, r0, csl)

        def att_full(c, h, lag=2):
            nt, r0 = h // 2, (h % 2) * 64
            csl = slice(c * 512, (c + 1) * 512)
            jmax = 4 * c + 3 if causal else LT - 1
            oaug = pso.tile([65, 512], F32, tag="oaug")

            def emit_o(jp, pt):
                for s in range(2):
                    j = 2 * jp + s
                    t = trim(c, j)
                    nc.tensor.matmul(
                        oaug[:, t:512], v_sb[j][:, h, :],
                        pt[:, s * 512 + t:(s + 1) * 512],
                        start=(j == 0), stop=(j == jmax))

            lagq = []
            for jp in range((jmax + 1) // 2):
                st = pst.tile([128, 1024], F32, tag="st")
                for s in range(2):
                    j = 2 * jp + s
                    t = trim(c, j)
                    nc.tensor.matmul(
                        st[:, s * 512 + t:(s + 1) * 512],
                        kt_sb[nt][r0:r0 + 64, j * 128:(j + 1) * 128],
                        qt_sb[nt][r0:r0 + 64, c * 512 + t:(c + 1) * 512],
                        start=True, stop=True)
                pt = p_pt.tile([128, 1024], BF16, tag="pt")
                t0 = trim(c, 2 * jp)
                nc.scalar.activation(pt[:, t0:], st[:, t0:], EXP,
                                     scale=exp_scale)
                if causal:
                    for s in range(2):
                        k = 2 * jp + s - 4 * c
                        if k >= 0:
                            sl = slice(s * 512 + 128 * k,
                                       s * 512 + 128 * (k + 1))
                            nc.vector.tensor_mul(pt[:, sl], pt[:, sl],
                                                 mk_t[:, :])
                lagq.append((jp, pt))
                if len(lagq) > lag:
                    emit_o(*lagq.pop(0))
            for args in lagq:
                emit_o(*args)
            normalize(oaug, nt, r0, csl)

        # ---- output projection pieces -----------------------------------
        def wo_piece(c, ots, tail=False):
            flush_yt()
            for ot in ots:
                ps = pp.tile([128, 512], F32, tag="pp")
                for kc2 in range(2):
                    nc.tensor.matmul(
                        ps[:, :], wo_sb[kc2][:, ot * 128:(ot + 1) * 128],
                        yt_sb[kc2][:, c * 512:(c + 1) * 512],
                        start=(kc2 == 0), stop=(kc2 == 1))
                oc = p_oc.tile([128, 512], F16, tag="oc")
                if tail:
                    nc.scalar.activation(oc[:, :], ps[:, :], COPY)
                else:
                    nc.vector.tensor_copy(oc[:, :], ps[:, :])
                nc.sync.dma_start(
                    out=outT.ap()[ot * 128:(ot + 1) * 128,
                                  c * 512:(c + 1) * 512],
                    in_=oc[:, :])

        # ---- emission schedule ------------------------------------------
        # exp stream starts as early as possible; V-dependent O work is
        # emitted after each chunk's S/exp block so late xt DMAs cannot
        # stall the in-order tensor queue ahead of the exps.
        proj_qk(0)
        # att(0): S/exp for all 4 heads first (8 pairs buffered in p_pt)
        pts0 = [att_se(0, h) for h in range(2)]
        proj_chunk(wq8_t, qt_sb, 1, "cosq", "sinq")
        pts0 += [att_se(0, h) for h in range(2, 4)]
        proj_chunk(wk8_t, kt_sb, 1, "cosk", "sink")
        for lt in range(4):
            v_tile(lt)
        for h in range(HPC):
            att_o(0, h, pts0[h])
        # att(1)
        pts10 = att_se(1, 0)
        for lt in range(4, 8):
            v_tile(lt)
        att_o(1, 0, pts10)
        proj_chunk(wq8_t, qt_sb, 2, "cosq", "sinq")
        att_full(1, 1)
        proj_chunk(wk8_t, kt_sb, 2, "cosk", "sink")
        att_full(1, 2)
        att_full(1, 3)
        # att(2)
        pts20 = att_se(2, 0)
        for lt in range(8, 12):
            v_tile(lt)
        att_o(2, 0, pts20)
        wo_piece(0, range(0, 4))
        att_full(2, 1)
        proj_chunk(wq8_t, qt_sb, 3, "cosq", "sinq")
        att_full(2, 2)
        proj_chunk(wk8_t, kt_sb, 3, "cosk", "sink")
        att_full(2, 3)
        wo_piece(0, range(4, 8))
        # att(3)
        pts30 = att_se(3, 0)
        for lt in range(12, 16):
            v_tile(lt)
        att_o(3, 0, pts30)
        wo_piece(1, range(0, 4))
        att_full(3, 1)
        wo_piece(1, range(4, 8))
        att_full(3, 2)
        wo_piece(2, range(0, 8))
        att_full(3, 3)
        wo_piece(3, range(8), tail=True)

    nc.compile()
    return nc


_EXP_SCALE = [1.0]


def _get_nc(causal: bool, exp_scale: float):
    key = ("causal" if causal else "dense", round(float(exp_scale), 12))
    if key not in _cache:
        _EXP_SCALE[0] = float(exp_scale)
        _cache[key] = _build_nc(causal)
    return _cache[key]


def _rope_np(x):
    d, s = x.shape[-1], x.shape[-2]
    ts = np.arange(0, d, 2, dtype=np.float32)
    inv = 10000.0 ** (-ts / d)
    grid = np.arange(s, dtype=np.float32)[:, None] * inv[None, :]
    sin = np.repeat(np.sin(grid), 2, axis=-1)
    cos = np.repeat(np.cos(grid), 2, axis=-1)
    x1, x2 = x[..., ::2], x[..., 1::2]
    xs = np.stack([-x2, x1], axis=-1).reshape(x.shape)
    return x * cos + xs * sin


def _reference_np(x, mask, Wq, Wk, Wv, Wo):
    b, l, d = x.shape
    h, k_sz = H, D // H
    split = lambda t: t.reshape(b, l, h, k_sz).transpose(0, 2, 1, 3)
    q = split((x @ Wq) / np.sqrt(np.float32(d)))
    q = _rope_np(q)
    k = _rope_np(split(x @ Wk))
    v = split(x @ Wv)
    logits = np.einsum("bhik,bhjk->bhij", q, k) + mask
    m = logits.max(axis=-1, keepdims=True)
    p = np.exp(logits - m)
    a = p / p.sum(axis=-1, keepdims=True)
    y = np.einsum("bhij,bhjv->bhiv", a, v)
    y = y.transpose(0, 2, 1, 3).reshape(b, l, d)
    return (y @ Wo).astype(np.float32)


def _spectral_norm(w, iters=12):
    rng = np.random.default_rng(0)
    v = rng.standard_normal(w.shape[1]).astype(np.float32)
    for _ in range(iters):
        u = w @ v
        u /= (np.linalg.norm(u) + 1e-30)
        v = w.T @ u
        nv = np.linalg.norm(v)
        v /= (nv + 1e-30)
    return float(nv)


def _host_consts():
    inv = 10000.0 ** (-np.arange(0, HD, 2, dtype=np.float32) / HD)
    grid = np.arange(L, dtype=np.float32)[None, :] * inv[:, None]   # [32, L]
    cos32 = np.cos(grid).astype(np.float32)
    sin32 = np.sin(grid).astype(np.float32)
    cos128 = np.ascontiguousarray(np.tile(cos32, (4, 1)))
    # srot rows u: u%64<32 (even-dim rows) get -sin, u%64>=32 get +sin
    srot128 = np.ascontiguousarray(
        np.tile(np.concatenate([-sin32, sin32], axis=0), (2, 1)))
    tri = (np.arange(128)[None, :] >= np.arange(128)[:, None]).astype(np.float32)
    return cos128, srot128, np.ascontiguousarray(tri)


def _pack_dr_w(Wc, Wc_s, scale, e4):
    """Wc/Wc_s [1024, 256] (straight / pair-swapped col orders) ->
    [128, 4096] fp8: col block (var*8 + kc2*2 + nt)*256 + plane*128 + m."""
    out = np.empty((128, 4096), np.float32)
    for var, W in ((0, Wc), (1, Wc_s)):
        for kc2 in range(KC2):
            for nt in range(2):
                off = (var * 8 + kc2 * 2 + nt) * 256
                blk = W[kc2 * 256:(kc2 + 1) * 256,
                        nt * 128:(nt + 1) * 128]  # [256, 128]
                out[:, off:off + 128] = blk[0:128]
                out[:, off + 128:off + 256] = blk[128:256]
    return np.clip(out * scale, -240, 240).astype(e4)


def _make_in_maps(x, Wq, Wk, Wv, Wo):
    import ml_dtypes
    bf16 = ml_dtypes.bfloat16
    e4 = ml_dtypes.float8_e4m3

    cos128, srot128, tri = _host_consts()

    sx = 240.0 / max(float(np.abs(x).max()), 1e-30)
    swq = 240.0 / max(float(np.abs(Wq).max()), 1e-30)
    swk = 240.0 / max(float(np.abs(Wk).max()), 1e-30)
    # fold the fp8 scales (and q's 1/sqrt(d_model)) into the rope tables so
    # qt/kt come out in true units and the exp scale is exactly 1
    fq = 1.0 / (sx * swq * float(np.sqrt(np.float32(D))))
    fk = 1.0 / (sx * swk)
    exp_scale = 1.0

    cosq = np.ascontiguousarray((cos128 * fq).astype(bf16))
    sinq = np.ascontiguousarray((srot128 * fq).astype(bf16))
    cosk = np.ascontiguousarray((cos128 * fk).astype(bf16))
    sink = np.ascontiguousarray((srot128 * fk).astype(bf16))
    mk4 = tri.astype(bf16)

    in_maps = []
    for core in range(NCORES):
        bi, g = core // 4, core % 4
        xTb = x[bi].T  # [1024, 2048] f32
        # fp8 DR x, packed per lc: [128, kc2*1024 + plane*512 + n]
        x8 = np.empty((4 * 128, 4096), np.float32)
        for lc in range(LC):
            lsl = slice(lc * 512, (lc + 1) * 512)
            r0 = lc * 128
            for kc2 in range(KC2):
                c0 = kc2 * 1024
                x8[r0:r0 + 128, c0:c0 + 512] = \
                    xTb[kc2 * 256:kc2 * 256 + 128, lsl]
                x8[r0:r0 + 128, c0 + 512:c0 + 1024] = \
                    xTb[kc2 * 256 + 128:kc2 * 256 + 256, lsl]
        x8 = np.clip(x8 * sx, -240, 240).astype(e4)

        # feature column orders: straight = per head [even dims | odd dims],
        # swapped = per head [odd dims | even dims] (rope pair partners)
        cols, cols_s = [], []
        for hh in range(HPC):
            base = (g * HPC + hh) * 64
            ev = list(range(base, base + 64, 2))
            od = list(range(base + 1, base + 64, 2))
            cols.extend(ev + od)
            cols_s.extend(od + ev)
        # wv packed: [128, kc*256 + v]
        wv_pk = np.empty((128, 8 * 256), np.float32)
        for kc in range(KC):
            wv_pk[:, kc * 256:(kc + 1) * 256] = \
                Wv[kc * 128:(kc + 1) * 128, g * 256:(g + 1) * 256]
        in_maps.append({
            "x8": x8,
            "xT": np.ascontiguousarray(xTb.astype(bf16)),
            "wq8": _pack_dr_w(Wq[:, cols], Wq[:, cols_s], swq, e4),
            "wk8": _pack_dr_w(Wk[:, cols], Wk[:, cols_s], swk, e4),
            "wv": np.ascontiguousarray(wv_pk.astype(bf16)),
            "wo": np.ascontiguousarray(
                Wo[g * 256:(g + 1) * 256, :].astype(bf16)),
            "cosq": cosq, "sinq": sinq, "cosk": cosk, "sink": sink,
            "mk4": mk4,
        })
    return in_maps, exp_scale


def kernel(x, mask, Wq, Wk, Wv, Wo):
    from concourse.bass_utils import run_bass_kernel_spmd

    x = np.asarray(x, dtype=np.float32)
    mask = np.asarray(mask, dtype=np.float32)
    Wq = np.asarray(Wq, dtype=np.float32)
    Wk = np.asarray(Wk, dtype=np.float32)
    Wv = np.asarray(Wv, dtype=np.float32)
    Wo = np.asarray(Wo, dtype=np.float32)

    m = mask.reshape(L, L)
    tril = np.tril(np.ones((L, L), dtype=bool))
    visible = m > -1e6
    if np.array_equal(visible, tril) and not m[tril].any():
        causal = True
    elif not m.any():
        causal = False
    else:
        return _reference_np(x, mask, Wq, Wk, Wv, Wo)

    # overflow guard for the no-max-subtraction softmax
    xr = float(np.sqrt((x * x).sum(axis=2).max()))
    bound = (xr * _spectral_norm(Wq) / np.sqrt(D)) * (xr * _spectral_norm(Wk))
    if bound > 60.0:
        return _reference_np(x, mask, Wq, Wk, Wv, Wo)

    in_maps, exp_scale = _make_in_maps(x, Wq, Wk, Wv, Wo)
    nc = _get_nc(causal, exp_scale)
    res = run_bass_kernel_spmd(nc, in_maps, core_ids=list(range(NCORES)))

    out = np.empty((B, L, D), dtype=np.float32)
    for bi in range(B):
        acc = res.results[bi * 4]["outT"].astype(np.float32)
        for g in range(1, 4):
            acc += res.results[bi * 4 + g]["outT"].astype(np.float32)
        out[bi] = acc.T
    return out


# revision 41
# speedup vs baseline: 1.0747x; 1.0042x over previous
"""Multi-head attention (16 heads, RoPE, causal) for Trainium2, 8 NeuronCores.

Sharding: data-parallel over batch (2) x tensor-parallel over head groups (4),
one (batch, head-group-of-4) pair per core. Each core computes its 4 heads'
attention feature-major and a partial output projection outT [1024, 2048] in
fp16; the host sums the 4 partials per batch and transposes back.

v2 highlights over the bf16 baseline:
  - Q/K projections in fp8e4 DoubleRow perf mode (2 contraction planes per
    matmul, 0.5 cycles/col): host packs x and Wq/Wk into [128, 2, N] paired
    layouts with absmax-derived scales.
  - Q^T/K^T stored fp8 in an even/odd paired layout ([128 partitions =
    4 heads x 32 pairs] x [2 planes: even|odd] x 2048), so the S^T matmul is
    also fp8 DoubleRow with the pair dim as the second contraction plane.
  - RoPE runs on DVE directly from the projection PSUM; the fp8/bf16
    quantization scales and 1/sqrt(d_model) are folded into per-path
    cos/sin constant tables, and 1/(sq*sk) into the exp() activation scale.
  - Scalar (ACT) engine runs exp() only -- it is the critical resource
    (~80us of 1 elem/cycle/partition work).
  - GpSimd handles SBUF-only work: causal tri-mask multiplies, softmax
    denominator partition-broadcasts, V ones-column memsets.
  - Inputs stream over three DMA queues (sync: x fp8, act: weights/consts,
    gpsimd: x bf16); output is fp16 (half the store traffic).
  - A short warm-up matmul block keeps the PE busy during the DMA ramp so
    the pstate governor is part-way up before real work lands.
"""

import sys

sys.path.insert(0, "/opt/trn_rl_repo")
sys.path.insert(0, "/root/.axon_site")

import numpy as np

B, L, D = 2, 2048, 1024
H = 16                  # total heads
HD = 64                 # head dim
HPC = 4                 # heads per core
NCORES = 8
LC = L // 512           # 512-wide l chunks
KC = D // 128           # 128-deep contraction chunks (bf16 V path)
KC2 = D // 256          # 256-deep DoubleRow contraction chunks
LT = L // 128           # 128-row l tiles
NWARM = 20              # PE warm-up matmuls

_cache = {}


def _build_nc(causal: bool):
    import contextlib

    import concourse.bass as bass
    import concourse.tile as tile
    from concourse import bacc, mybir

    F32 = mybir.dt.float32
    BF16 = mybir.dt.bfloat16
    FP8 = mybir.dt.float8e4
    F16 = mybir.dt.float16
    EXP = mybir.ActivationFunctionType.Exp
    COPY = mybir.ActivationFunctionType.Copy
    DR = mybir.MatmulPerfMode.DoubleRow

    nc = bacc.Bacc("TRN2", target_bir_lowering=False, debug=False, num_devices=NCORES)

    # x fp8 packed per lc chunk: [128, kc2*1024 + plane*512 + n] (4KB rows)
    x8 = nc.dram_tensor("x8", [4 * 128, 4096], FP8, kind="ExternalInput")
    xT = nc.dram_tensor("xT", [D, L], BF16, kind="ExternalInput")
    # all Q (or K) DoubleRow weights in one 4KB-row tensor:
    # col block (variant*8 + kc2*2 + nt)*256 + plane*128 + m
    # variant 0 = straight feature order, 1 = pair-swapped columns so
    # rope is rot = cosq*P + srotq*Ps (no on-chip partition swap needed)
    wq8 = nc.dram_tensor("wq8", [128, 4096], FP8, kind="ExternalInput")
    wk8 = nc.dram_tensor("wk8", [128, 4096], FP8, kind="ExternalInput")
    # wv packed: col block kc*256 + v  (4KB rows bf16)
    wv = nc.dram_tensor("wv", [128, 8 * 256], BF16, kind="ExternalInput")
    wo = nc.dram_tensor("wo", [256, D], BF16, kind="ExternalInput")
    cosr = nc.dram_tensor("cosr", [128, L], BF16, kind="ExternalInput")
    srot = nc.dram_tensor("srot", [128, L], BF16, kind="ExternalInput")
    mk4 = nc.dram_tensor("mk4", [128, 128], BF16, kind="ExternalInput")
    outT = nc.dram_tensor("outT", [D, L], F16, kind="ExternalOutput")

    with tile.TileContext(nc) as tc, \
         nc.allow_low_precision(reason="fp8/bf16 matmul pipeline by design"), \
         contextlib.ExitStack() as ctx:
        p_w8 = ctx.enter_context(tc.tile_pool(name="p_w8", bufs=2))
        p_wv = ctx.enter_context(tc.tile_pool(name="p_wv", bufs=1))
        p_wo = ctx.enter_context(tc.tile_pool(name="p_wo", bufs=2))
        p_const = ctx.enter_context(tc.tile_pool(name="p_const", bufs=6))
        p_x8 = ctx.enter_context(tc.tile_pool(name="p_x8", bufs=4))
        p_xt = ctx.enter_context(tc.tile_pool(name="p_xt", bufs=8))
        p_qk = ctx.enter_context(tc.tile_pool(name="p_qk", bufs=4))
        p_yt = ctx.enter_context(tc.tile_pool(name="p_yt", bufs=2))
        p_v = ctx.enter_context(tc.tile_pool(name="p_v", bufs=16))
        p_pt = ctx.enter_context(tc.tile_pool(name="p_pt", bufs=13))
        p_tmp = ctx.enter_context(tc.tile_pool(name="p_tmp", bufs=6))
        p_zs = ctx.enter_context(tc.tile_pool(name="p_zs", bufs=4))
        p_zb = ctx.enter_context(tc.tile_pool(name="p_zb", bufs=3))
        p_oc = ctx.enter_context(tc.tile_pool(name="p_oc", bufs=6))
        p_wu = ctx.enter_context(tc.tile_pool(name="p_wu", bufs=1))
        pp = ctx.enter_context(tc.tile_pool(name="pp", bufs=2, space="PSUM"))
        pst = ctx.enter_context(tc.tile_pool(name="pst", bufs=2, space="PSUM"))
        pso = ctx.enter_context(tc.tile_pool(name="pso", bufs=2, space="PSUM"))

        # ---- warm-up: keep PE busy while DMAs land (pstate ramp).
        # Quarter-density (Pout=32) so the power-credit pool isn't drained
        # before the real work starts.
        wu = p_wu.tile([128, 512], BF16, tag="wu")
        nc.vector.memset(wu[:, :], 0.125)
        wu_ps = pp.tile([128, 512], F32, tag="pp")
        for _ in range(NWARM):
            nc.tensor.matmul(wu_ps[0:32, :], wu[:, 0:32], wu[:, :],
                             start=True, stop=True)

        # ---- input DMAs over two queues (ACT issues none: it is the
        # critical exp engine). Q-path inputs on sync, K-path on gpsimd so
        # the first S matmul's dependencies land in parallel. All inputs
        # are packed host-side into 4KB-row tensors for fat descriptors. --
        x8_sb = {}

        def load_x8(lc, eng):
            t = p_x8.tile([128, 4096], FP8, tag="x8", name=f"x8_{lc}")
            eng.dma_start(out=t, in_=x8.ap()[lc * 128:(lc + 1) * 128, :])
            x8_sb[lc] = t

        rope_c = {}

        def load_const(nm, src, eng):
            t = p_const.tile([128, L], BF16, tag="const")
            eng.dma_start(out=t, in_=src.ap())
            rope_c[nm] = t

        xt_sb = [p_xt.tile([128, L], BF16, tag="xt", name=f"xt{kc}")
                 for kc in range(KC)]

        def load_xt(kc, eng):
            eng.dma_start(out=xt_sb[kc],
                          in_=xT.ap()[kc * 128:(kc + 1) * 128, :])

        # sync queue (Q path first, then V-path x)
        load_x8(0, nc.sync)
        wq8_t = p_w8.tile([128, 4096], FP8, tag="w8")
        nc.sync.dma_start(out=wq8_t, in_=wq8.ap())
        load_const("cos", cosr, nc.sync)
        load_x8(1, nc.sync)
        for kc in range(4):
            load_xt(kc, nc.sync)
        wv_t = p_wv.tile([128, 8 * 256], BF16, tag="wv")
        nc.sync.dma_start(out=wv_t, in_=wv.ap())
        # gpsimd queue (K path first, then remaining V-path x)
        mk_t = p_const.tile([128, 128], BF16, tag="tri")
        nc.gpsimd.dma_start(out=mk_t, in_=mk4.ap())
        wk8_t = p_w8.tile([128, 4096], FP8, tag="w8")
        nc.gpsimd.dma_start(out=wk8_t, in_=wk8.ap())
        load_const("srot", srot, nc.gpsimd)
        for kc in range(4, KC):
            load_xt(kc, nc.gpsimd)
        load_x8(2, nc.gpsimd)
        load_x8(3, nc.gpsimd)
        wo_sb = []
        for kc2 in range(2):
            t = p_wo.tile([128, D], BF16, tag="wo")
            nc.gpsimd.dma_start(out=t, in_=wo.ap()[kc2 * 128:(kc2 + 1) * 128, :])
            wo_sb.append(t)

        # persistent activation tiles: bf16 Q^T/K^T, 2 heads per nt tile,
        # rows h*64+u with u<32 = even rotary dims, u>=32 = odd dims
        qt_sb = [p_qk.tile([128, L], BF16, tag="qt", name=f"qt{i}")
                 for i in range(2)]
        kt_sb = [p_qk.tile([128, L], BF16, tag="kt", name=f"kt{i}")
                 for i in range(2)]
        yt_sb = [p_yt.tile([128, L], BF16, tag="yt", name=f"yt{i}")
                 for i in range(2)]
        v_sb = [p_v.tile([128, HPC, 65], BF16, tag="vaug", name=f"vaug{i}")
                for i in range(LT)]
        for lt in range(LT):
            nc.gpsimd.memset(v_sb[lt][:, :, 64:65], 1.0)

        # ---- QK projection (fp8 DoubleRow x2) + RoPE -------------------
        # Two parallel projections per tile: P = x@W (straight features)
        # and Ps = x@Ws (pair-swapped features), then
        # rope(q) = cos*P + srot*Ps with per-row-signed srot.
        def proj_chunk(w_t, trg, lc, cosn, sinn):
            csl = slice(lc * 512, (lc + 1) * 512)
            cos_t, sin_t = rope_c[cosn], rope_c[sinn]
            for nt in range(2):
                ps1 = pp.tile([128, 512], F32, tag="pp")
                ps2 = pp.tile([128, 512], F32, tag="pp")
                for dst, var in ((ps1, 0), (ps2, 1)):
                    for kc2 in range(KC2):
                        woff = (var * 8 + kc2 * 2 + nt) * 256
                        nc.tensor.matmul(
                            dst[:, :],
                            w_t[:, woff:woff + 256].rearrange(
                                "p (two m) -> p two m", two=2),
                            x8_sb[lc][:, kc2 * 1024:(kc2 + 1) * 1024
                                      ].rearrange(
                                "p (two n) -> p two n", two=2),
                            start=(kc2 == 0), stop=(kc2 == KC2 - 1),
                            perf_mode=DR)
                m1 = p_tmp.tile([128, 512], BF16, tag="tmp")
                nc.vector.tensor_mul(m1[:, :], ps1[:, :], cos_t[:, csl])
                m2 = p_tmp.tile([128, 512], BF16, tag="tmp")
                nc.vector.tensor_mul(m2[:, :], ps2[:, :], sin_t[:, csl])
                nc.vector.tensor_add(trg[nt][:, csl], m1[:, :], m2[:, :])

        def proj_qk(lc):
            proj_chunk(wq8_t, qt_sb, lc, "cos", "srot")
            proj_chunk(wk8_t, kt_sb, lc, "cos", "srot")

        # ---- V tile (bf16) ---------------------------------------------
        def v_tile(lt):
            ps = pp.tile([128, 256], F32, tag="pp")
            for kc in range(KC):
                nc.tensor.matmul(
                    ps[:, :], xt_sb[kc][:, lt * 128:(lt + 1) * 128],
                    wv_t[:, kc * 256:(kc + 1) * 256],
                    start=(kc == 0), stop=(kc == KC - 1))
            nc.vector.tensor_copy(
                v_sb[lt][:, :, 0:64],
                ps[:, :].rearrange("p (h v) -> p h v", h=HPC))

        # ---- attention -------------------------------------------------
        exp_scale = float(_EXP_SCALE[0])
        pending_yt = []   # deferred normalize muls (DVE must not stall on
                          # the gpsimd broadcast latency)

        def flush_yt():
            while pending_yt:
                oaug, zb, nt, r0, csl = pending_yt.pop(0)
                nc.vector.tensor_mul(yt_sb[nt][r0:r0 + 64, csl],
                                     oaug[0:64, :], zb[:, :])

        def normalize(oaug, nt, r0, csl):
            zs = p_zs.tile([1, 512], F32, tag="zs")
            nc.vector.tensor_copy(zs[0:1, :], oaug[64:65, :])
            zrow = p_zs.tile([1, 512], F32, tag="zrow")
            nc.vector.reciprocal_approx_fast(zrow[0:1, :], zs[0:1, :])
            zb = p_zb.tile([64, 512], F32, tag="zb")
            nc.gpsimd.partition_broadcast(zb[:, :], zrow[0:1, :])
            flush_yt()
            pending_yt.append((oaug, zb, nt, r0, csl))

        def trim(c, j):
            k = j - 4 * c
            return 128 * k if (causal and k >= 0) else 0

        def att_se(c, h):
            """S matmuls + exp for all pairs of (c, h); returns pt list."""
            nt, r0 = h // 2, (h % 2) * 64
            jmax = 4 * c + 3 if causal else LT - 1
            pts = []
            for jp in range((jmax + 1) // 2):
                st = pst.tile([128, 1024], F32, tag="st")
                for s in range(2):
                    j = 2 * jp + s
                    t = trim(c, j)
                    nc.tensor.matmul(
                        st[:, s * 512 + t:(s + 1) * 512],
                        kt_sb[nt][r0:r0 + 64, j * 128:(j + 1) * 128],
                        qt_sb[nt][r0:r0 + 64, c * 512 + t:(c + 1) * 512],
                        start=True, stop=True)
                pt = p_pt.tile([128, 1024], BF16, tag="pt")
                t0 = trim(c, 2 * jp)
                nc.scalar.activation(pt[:, t0:], st[:, t0:], EXP,
                                     scale=exp_scale)
                if causal:
                    for s in range(2):
                        k = 2 * jp + s - 4 * c
                        if k >= 0:
                            sl = slice(s * 512 + 128 * k,
                                       s * 512 + 128 * (k + 1))
                            nc.vector.tensor_mul(pt[:, sl], pt[:, sl],
                                                 mk_t[:, :])
                pts.append((jp, pt))
            return pts

        def att_o(c, h, pts):
            """O accumulation + raw evac + recip/broadcast; defers yt mul."""
            nt, r0 = h // 2, (h % 2) * 64
            csl = slice(c * 512, (c + 1) * 512)
            jmax = 4 * c + 3 if causal else LT - 1
            oaug = pso.tile([65, 512], F32, tag="oaug")
            for jp, pt in pts:
                for s in range(2):
                    j = 2 * jp + s
                    t = trim(c, j)
                    nc.tensor.matmul(
                        oaug[:, t:512], v_sb[j][:, h, :],
                        pt[:, s * 512 + t:(s + 1) * 512],
                        start=(j == 0), stop=(j == jmax))
            normalize(oaug, nt, r0, csl)

        def att_full(c, h, lag=2):
            nt, r0 = h // 2, (h % 2) * 64
            csl = slice(c * 512, (c + 1) * 512)
            jmax = 4 * c + 3 if causal else LT - 1
            oaug = pso.tile([65, 512], F32, tag="oaug")

            def emit_o(jp, pt):
                for s in range(2):
                    j = 2 * jp + s
                    t = trim(c, j)
                    nc.tensor.matmul(
                        oaug[:, t:512], v_sb[j][:, h, :],
                        pt[:, s * 512 + t:(s + 1) * 512],
                        start=(j == 0), stop=(j == jmax))

            lagq = []
            for jp in range((jmax + 1) // 2):
                st = pst.tile([128, 1024], F32, tag="st")
                for s in range(2):
                    j = 2 * jp + s
                    t = trim(c, j)
                    nc.tensor.matmul(
                        st[:, s * 512 + t:(s + 1) * 512],
                        kt_sb[nt][r0:r0 + 64, j * 128:(j + 1) * 128],
                        qt_sb[nt][r0:r0 + 64, c * 512 + t:(c + 1) * 512],
                        start=True, stop=True)
                pt = p_pt.tile([128, 1024], BF16, tag="pt")
                t0 = trim(c, 2 * jp)
                nc.scalar.activation(pt[:, t0:], st[:, t0:], EXP,
                                     scale=exp_scale)
                if causal:
                    for s in range(2):
                        k = 2 * jp + s - 4 * c
                        if k >= 0:
                            sl = slice(s * 512 + 128 * k,
                                       s * 512 + 128 * (k + 1))
                            nc.vector.tensor_mul(pt[:, sl], pt[:, sl],
                                                 mk_t[:, :])
                lagq.append((jp, pt))
                if len(lagq) > lag:
                    emit_o(*lagq.pop(0))
            for args in lagq:
                emit_o(*args)
            normalize(oaug, nt, r0, csl)

        # ---- output projection pieces -----------------------------------
        def wo_piece(c, ots, tail=False):
            flush_yt()
            for ot in ots:
                ps = pp.tile([128, 512], F32, tag="pp")
                for kc2 in range(2):
                    nc.tensor.matmul(
                        ps[:, :], wo_sb[kc2][:, ot * 128:(ot + 1) * 128],
                        yt_sb[kc2][:, c * 512:(c + 1) * 512],
                        start=(kc2 == 0), stop=(kc2 == 1))
                oc = p_oc.tile([128, 512], F16, tag="oc")
                if tail:
                    nc.scalar.activation(oc[:, :], ps[:, :], COPY)
                else:
                    nc.vector.tensor_copy(oc[:, :], ps[:, :])
                nc.sync.dma_start(
                    out=outT.ap()[ot * 128:(ot + 1) * 128,
                                  c * 512:(c + 1) * 512],
                    in_=oc[:, :])

        # ---- emission schedule ------------------------------------------
        # exp stream starts as early as possible; V-dependent O work is
        # emitted after each chunk's S/exp block so late xt DMAs cannot
        # stall the in-order tensor queue ahead of the exps.
        proj_qk(0)
        # att(0): S/exp for all 4 heads first (8 pairs buffered in p_pt)
        pts0 = [att_se(0, h) for h in range(2)]
        proj_chunk(wq8_t, qt_sb, 1, "cos", "srot")
        pts0 += [att_se(0, h) for h in range(2, 4)]
        proj_chunk(wk8_t, kt_sb, 1, "cos", "srot")
        for lt in range(4):
            v_tile(lt)
        for h in range(HPC):
            att_o(0, h, pts0[h])
        # att(1)
        pts10 = att_se(1, 0)
        for lt in range(4, 8):
            v_tile(lt)
        att_o(1, 0, pts10)
        proj_chunk(wq8_t, qt_sb, 2, "cos", "srot")
        att_full(1, 1)
        proj_chunk(wk8_t, kt_sb, 2, "cos", "srot")
        att_full(1, 2)
        att_full(1, 3)
        # att(2)
        pts20 = att_se(2, 0)
        for lt in range(8, 12):
            v_tile(lt)
        att_o(2, 0, pts20)
        wo_piece(0, range(0, 4))
        att_full(2, 1)
        proj_chunk(wq8_t, qt_sb, 3, "cos", "srot")
        att_full(2, 2)
        proj_chunk(wk8_t, kt_sb, 3, "cos", "srot")
        att_full(2, 3)
        wo_piece(0, range(4, 8))
        # att(3)
        pts30 = att_se(3, 0)
        for lt in range(12, 16):
            v_tile(lt)
        att_o(3, 0, pts30)
        wo_piece(1, range(0, 4))
        att_full(3, 1)
        wo_piece(1, range(4, 8))
        att_full(3, 2)
        wo_piece(2, range(0, 8))
        att_full(3, 3)
        wo_piece(3, range(8), tail=True)

    nc.compile()
    return nc


_EXP_SCALE = [1.0]


def _get_nc(causal: bool, exp_scale: float):
    key = ("causal" if causal else "dense", round(float(exp_scale), 12))
    if key not in _cache:
        _EXP_SCALE[0] = float(exp_scale)
        _cache[key] = _build_nc(causal)
    return _cache[key]


def _rope_np(x):
    d, s = x.shape[-1], x.shape[-2]
    ts = np.arange(0, d, 2, dtype=np.float32)
    inv = 10000.0 ** (-ts / d)
    grid = np.arange(s, dtype=np.float32)[:, None] * inv[None, :]
    sin = np.repeat(np.sin(grid), 2, axis=-1)
    cos = np.repeat(np.cos(grid), 2, axis=-1)
    x1, x2 = x[..., ::2], x[..., 1::2]
    xs = np.stack([-x2, x1], axis=-1).reshape(x.shape)
    return x * cos + xs * sin


def _reference_np(x, mask, Wq, Wk, Wv, Wo):
    b, l, d = x.shape
    h, k_sz = H, D // H
    split = lambda t: t.reshape(b, l, h, k_sz).transpose(0, 2, 1, 3)
    q = split((x @ Wq) / np.sqrt(np.float32(d)))
    q = _rope_np(q)
    k = _rope_np(split(x @ Wk))
    v = split(x @ Wv)
    logits = np.einsum("bhik,bhjk->bhij", q, k) + mask
    m = logits.max(axis=-1, keepdims=True)
    p = np.exp(logits - m)
    a = p / p.sum(axis=-1, keepdims=True)
    y = np.einsum("bhij,bhjv->bhiv", a, v)
    y = y.transpose(0, 2, 1, 3).reshape(b, l, d)
    return (y @ Wo).astype(np.float32)


def _spectral_norm(w, iters=12):
    rng = np.random.default_rng(0)
    v = rng.standard_normal(w.shape[1]).astype(np.float32)
    for _ in range(iters):
        u = w @ v
        u /= (np.linalg.norm(u) + 1e-30)
        v = w.T @ u
        nv = np.linalg.norm(v)
        v /= (nv + 1e-30)
    return float(nv)


def _host_consts():
    inv = 10000.0 ** (-np.arange(0, HD, 2, dtype=np.float32) / HD)
    grid = np.arange(L, dtype=np.float32)[None, :] * inv[:, None]   # [32, L]
    cos32 = np.cos(grid).astype(np.float32)
    sin32 = np.sin(grid).astype(np.float32)
    cos128 = np.ascontiguousarray(np.tile(cos32, (4, 1)))
    # srot rows u: u%64<32 (even-dim rows) get -sin, u%64>=32 get +sin
    srot128 = np.ascontiguousarray(
        np.tile(np.concatenate([-sin32, sin32], axis=0), (2, 1)))
    tri = (np.arange(128)[None, :] >= np.arange(128)[:, None]).astype(np.float32)
    return cos128, srot128, np.ascontiguousarray(tri)


def _pack_dr_w(Wc, Wc_s, scale, e4):
    """Wc/Wc_s [1024, 256] (straight / pair-swapped col orders) ->
    [128, 4096] fp8: col block (var*8 + kc2*2 + nt)*256 + plane*128 + m."""
    out = np.empty((128, 4096), np.float32)
    for var, W in ((0, Wc), (1, Wc_s)):
        for kc2 in range(KC2):
            for nt in range(2):
                off = (var * 8 + kc2 * 2 + nt) * 256
                blk = W[kc2 * 256:(kc2 + 1) * 256,
                        nt * 128:(nt + 1) * 128]  # [256, 128]
                out[:, off:off + 128] = blk[0:128]
                out[:, off + 128:off + 256] = blk[128:256]
    return np.clip(out * scale, -240, 240).astype(e4)


def _make_in_maps(x, Wq, Wk, Wv, Wo):
    import ml_dtypes
    bf16 = ml_dtypes.bfloat16
    e4 = ml_dtypes.float8_e4m3

    cos128, srot128, tri = _host_consts()

    sx = 240.0 / max(float(np.abs(x).max()), 1e-30)
    swq = 240.0 / max(float(np.abs(Wq).max()), 1e-30)
    swk = 240.0 / max(float(np.abs(Wk).max()), 1e-30)
    # rope tables are plain cos/srot shared by Q and K; all fp8 scales and
    # q's 1/sqrt(d_model) cancel inside the exp() activation scale
    exp_scale = 1.0 / (sx * sx * swq * swk * float(np.sqrt(np.float32(D))))

    cosr = np.ascontiguousarray(cos128.astype(bf16))
    srot = np.ascontiguousarray(srot128.astype(bf16))
    mk4 = tri.astype(bf16)

    in_maps = []
    for core in range(NCORES):
        bi, g = core // 4, core % 4
        xTb = x[bi].T  # [1024, 2048] f32
        # fp8 DR x, packed per lc: [128, kc2*1024 + plane*512 + n]
        x8 = np.empty((4 * 128, 4096), np.float32)
        for lc in range(LC):
            lsl = slice(lc * 512, (lc + 1) * 512)
            r0 = lc * 128
            for kc2 in range(KC2):
                c0 = kc2 * 1024
                x8[r0:r0 + 128, c0:c0 + 512] = \
                    xTb[kc2 * 256:kc2 * 256 + 128, lsl]
                x8[r0:r0 + 128, c0 + 512:c0 + 1024] = \
                    xTb[kc2 * 256 + 128:kc2 * 256 + 256, lsl]
        x8 = np.clip(x8 * sx, -240, 240).astype(e4)

        # feature column orders: straight = per head [even dims | odd dims],
        # swapped = per head [odd dims | even dims] (rope pair partners)
        cols, cols_s = [], []
        for hh in range(HPC):
            base = (g * HPC + hh) * 64
            ev = list(range(base, base + 64, 2))
            od = list(range(base + 1, base + 64, 2))
            cols.extend(ev + od)
            cols_s.extend(od + ev)
        # wv packed: [128, kc*256 + v]
        wv_pk = np.empty((128, 8 * 256), np.float32)
        for kc in range(KC):
            wv_pk[:, kc * 256:(kc + 1) * 256] = \
                Wv[kc * 128:(kc + 1) * 128, g * 256:(g + 1) * 256]
        in_maps.append({
            "x8": x8,
            "xT": np.ascontiguousarray(xTb.astype(bf16)),
            "wq8": _pack_dr_w(Wq[:, cols], Wq[:, cols_s], swq, e4),
            "wk8": _pack_dr_w(Wk[:, cols], Wk[:, cols_s], swk, e4),
            "wv": np.ascontiguousarray(wv_pk.astype(bf16)),
            "wo": np.ascontiguousarray(
                Wo[g * 256:(g + 1) * 256, :].astype(bf16)),
            "cosr": cosr, "srot": srot, "mk4": mk4,
        })
    return in_maps, exp_scale


def kernel(x, mask, Wq, Wk, Wv, Wo):
    from concourse.bass_utils import run_bass_kernel_spmd

    x = np.asarray(x, dtype=np.float32)
    mask = np.asarray(mask, dtype=np.float32)
    Wq = np.asarray(Wq, dtype=np.float32)
    Wk = np.asarray(Wk, dtype=np.float32)
    Wv = np.asarray(Wv, dtype=np.float32)
    Wo = np.asarray(Wo, dtype=np.float32)

    m = mask.reshape(L, L)
    tril = np.tril(np.ones((L, L), dtype=bool))
    visible = m > -1e6
    if np.array_equal(visible, tril) and not m[tril].any():
        causal = True
    elif not m.any():
        causal = False
    else:
        return _reference_np(x, mask, Wq, Wk, Wv, Wo)

    # overflow guard for the no-max-subtraction softmax
    xr = float(np.sqrt((x * x).sum(axis=2).max()))
    bound = (xr * _spectral_norm(Wq) / np.sqrt(D)) * (xr * _spectral_norm(Wk))
    if bound > 60.0:
        return _reference_np(x, mask, Wq, Wk, Wv, Wo)

    in_maps, exp_scale = _make_in_maps(x, Wq, Wk, Wv, Wo)
    nc = _get_nc(causal, exp_scale)
    res = run_bass_kernel_spmd(nc, in_maps, core_ids=list(range(NCORES)))

    out = np.empty((B, L, D), dtype=np.float32)
    for bi in range(B):
        acc = res.results[bi * 4]["outT"].astype(np.float32)
        for g in range(1, 4):
            acc += res.results[bi * 4 + g]["outT"].astype(np.float32)
        out[bi] = acc.T
    return out
